# revision 1
# baseline (speedup 1.0000x reference)
"""MeshMeanFlowNet block on 8 Trainium2 NeuronCores.

Sharding: data-parallel over B (one batch element per core), no collectives.
All activations are kept feature-major on device ([feature, token]) so every
linear layer consumes its input directly as the matmul moving operand and
produces feature-major output. The attention softmax is computed in the
transposed layout S^T[j, i] (j = key token on partitions, i = query token on
the free axis); the softmax denominator comes for free from a ones-row
appended to V, so no on-device transposes are needed anywhere: V is produced
token-major by swapping the matmul operand roles for the v-part of the qkv
projection.

The per-edge-type/per-head bias never materializes a gather: with only 4 edge
types, softmax shift-invariance removes type 3, and the remaining three
(table[e,h] - table[3,h]) deltas are applied as three fused
(mask_e * c_eh + acc) scalar_tensor_tensor ops on top of the score PSUM,
where mask_e = (edge == e) is shared across all heads.
"""

import sys

sys.path.insert(0, "/opt/trn_rl_repo")

import ml_dtypes
import numpy as np

B, V, D, H = 8, 1024, 512, 8
HD = D // H  # 64
NCORES = 8

_cache = {}


def _build_program(cb, probe=False):
    """cb: [3][H] float bias deltas (edge_table[e,h] - edge_table[3,h])."""
    import contextlib

    import concourse.bacc as bacc
    import concourse.tile as tile
    from concourse import mybir

    f32 = mybir.dt.float32
    f32r = mybir.dt.float32r
    bf16 = mybir.dt.bfloat16
    ALU = mybir.AluOpType
    ACTF = mybir.ActivationFunctionType

    nc = bacc.Bacc("TRN2", target_bir_lowering=False, debug=False,
                   num_devices=NCORES)

    # ---- DRAM I/O (per-core shard, host pre-laid-out) ----
    xT = nc.dram_tensor("xT", [D, V], f32r, kind="ExternalInput")
    eiT = nc.dram_tensor("eiT", [V, V], bf16, kind="ExternalInput")  # [j, i]
    condc = nc.dram_tensor("condc", [4, 128], f32, kind="ExternalInput")
    wqk = nc.dram_tensor("wqk", [D, 1024], f32r, kind="ExternalInput")
    wv = nc.dram_tensor("wv", [D, 512], f32r, kind="ExternalInput")
    wada = nc.dram_tensor("wada", [D, 2048], f32r, kind="ExternalInput")
    bada = nc.dram_tensor("bada", [16, 128], f32, kind="ExternalInput")
    wproj = nc.dram_tensor("wproj", [D, D], f32r, kind="ExternalInput")
    bproj = nc.dram_tensor("bproj", [4, 128], f32, kind="ExternalInput")
    wm1 = nc.dram_tensor("wm1", [D, 2048], f32r, kind="ExternalInput")
    bm1 = nc.dram_tensor("bm1", [16, 128], f32, kind="ExternalInput")
    wm2 = nc.dram_tensor("wm2", [2048, D], f32r, kind="ExternalInput")
    bm2 = nc.dram_tensor("bm2", [4, 128], f32, kind="ExternalInput")
    onesc = nc.dram_tensor("onesc", [128, 8], f32r, kind="ExternalInput")
    onesb = nc.dram_tensor("onesb", [128, 8], bf16, kind="ExternalInput")
    yT = nc.dram_tensor("yT", [D, V], f32, kind="ExternalOutput")
    if probe:
        p_params = nc.dram_tensor("p_params", [128, 16], f32, kind="ExternalOutput")
        p_h1 = nc.dram_tensor("p_h1", [D, V], f32, kind="ExternalOutput")
        p_qk = nc.dram_tensor("p_qk", [8, 128, V], f32, kind="ExternalOutput")
        p_vaug = nc.dram_tensor("p_vaug", [8, 128, 8, 65], f32, kind="ExternalOutput")
        p_att = nc.dram_tensor("p_att", [D, V], f32, kind="ExternalOutput")
        p_x2 = nc.dram_tensor("p_x2", [D, V], f32, kind="ExternalOutput")
        p_h2 = nc.dram_tensor("p_h2", [D, V], f32, kind="ExternalOutput")
        p_L = nc.dram_tensor("p_L", [128, 512], bf16, kind="ExternalOutput")
        p_P = nc.dram_tensor("p_P", [128, 512], bf16, kind="ExternalOutput")
        p_m0 = nc.dram_tensor("p_m0", [128, 512], bf16, kind="ExternalOutput")
        p_ops = nc.dram_tensor("p_ops", [65, 512], f32, kind="ExternalOutput")

    def mm(out, lhsT, rhs, **kw):
        nc.tensor.matmul(out, lhsT.bitcast(f32r), rhs.bitcast(f32r), **kw)

    with tile.TileContext(nc) as tc:
        with contextlib.ExitStack() as ctx:
            # ---- whole-program pool: ~60KB/partition ----
            persist = ctx.enter_context(tc.tile_pool(name="persist", bufs=1))

            ones = persist.tile([128, 1], f32r, tag="ones")
            nc.sync.dma_start(out=ones, in_=onesc[:, 0:1])
            epst = persist.tile([1, 1], f32, tag="eps")
            nc.vector.memset(epst, 1e-5)

            # x (feature-major)
            xT_t = [persist.tile([128, V], f32r, tag=f"xT{kc}",
                                 name=f"xT_t{kc}") for kc in range(4)]
            for kc in range(4):
                nc.sync.dma_start(out=xT_t[kc],
                                  in_=xT[kc * 128:(kc + 1) * 128, :])

            bada_t = persist.tile([128, 16], f32, tag="bada")
            nc.sync.dma_start(out=bada_t, in_=bada[:].rearrange("c p -> p c"))
            bp_t = persist.tile([128, 4], f32, tag="bproj")
            nc.sync.dma_start(out=bp_t, in_=bproj[:].rearrange("c p -> p c"))
            bm1_t = persist.tile([128, 16], f32, tag="bm1")
            nc.sync.dma_start(out=bm1_t, in_=bm1[:].rearrange("c p -> p c"))
            bm2_t = persist.tile([128, 4], f32, tag="bm2")
            nc.sync.dma_start(out=bm2_t, in_=bm2[:].rearrange("c p -> p c"))

            x2 = [persist.tile([128, V], f32r, tag=f"x2_{kc}",
                                name=f"x2_{kc}") for kc in range(4)]
            params = persist.tile([128, 16], f32, tag="params")
            nparams = persist.tile([128, 8], f32, tag="nparams")

            # ---------- AdaLN parameter path ----------
            with tc.tile_pool(name="adaw", bufs=1) as adaw, \
                    tc.tile_pool(name="adap", bufs=2, space="PSUM") as adap:
                condt = adaw.tile([128, 5], f32, tag="cond")
                nc.sync.dma_start(out=condt[:, 0:4],
                                  in_=condc[:].rearrange("c p -> p c"))
                nc.vector.memset(condt[:, 4:5], 0.0)
                scond = adaw.tile([128, 5], f32r, tag="scond")
                nc.scalar.activation(scond, condt, ACTF.Silu)
                wada_t = [adaw.tile([128, 2048], f32r, tag=f"wada{kc}",
                                    name="wada_t") for kc in range(4)]
                for kc in range(4):
                    nc.sync.dma_start(out=wada_t[kc],
                                      in_=wada[kc * 128:(kc + 1) * 128, :])
                pp = adap.tile([2, 2048], f32, tag="pada")
                for oc in range(4):
                    s = slice(oc * 512, oc * 512 + 512)
                    for kc in range(4):
                        mm(pp[:, s], scond[:, kc:kc + 2], wada_t[kc][:, s],
                           start=(kc == 0), stop=(kc == 3))
                prow = adaw.tile([1, 2048], f32, tag="prow")
                nc.scalar.copy(prow, pp[0:1, :])
                pscat = adaw.tile([128, 16], f32, tag="pscat")
                for md in range(16):
                    nc.sync.dma_start(
                        out=pscat[:, md:md + 1],
                        in_=prow[0:1, md * 128:(md + 1) * 128])
                nc.vector.tensor_add(params, pscat, bada_t)
                for ln in range(2):
                    for kc in range(4):
                        sc = ln * 8 + kc
                        nc.vector.tensor_scalar(
                            nparams[:, ln * 4 + kc:ln * 4 + kc + 1],
                            params[:, sc:sc + 1], -1.0, None, ALU.mult)
                if probe:
                    nc.sync.dma_start(out=p_params[:], in_=params)

            def adaln(src_tiles, ln_idx, dst_pool, out_tag):
                """LayerNorm over the partition (feature) axis + adaptive
                affine from `params`. Returns 4 feature-major tiles."""
                out = [dst_pool.tile([128, V], f32r, tag=f"{out_tag}{kc}",
                                     name=f"ln_{out_tag}{kc}")
                       for kc in range(4)]
                with tc.tile_pool(name="lnt", bufs=1) as lnt, \
                        tc.tile_pool(name="lnp", bufs=1,
                                     space="PSUM") as lnp:
                    ps_s = lnp.tile([1, V], f32, tag="lnsum")
                    ps_q = lnp.tile([1, V], f32, tag="lnsqsum")
                    for kc in range(4):
                        sq = lnt.tile([128, V], f32r, tag="lnsq", bufs=2,
                                      name="sq")
                        nc.scalar.square(sq, src_tiles[kc].bitcast(f32))
                        for nh in range(2):
                            s = slice(nh * 512, nh * 512 + 512)
                            mm(ps_s[:, s], ones, src_tiles[kc][:, s],
                               start=(kc == 0), stop=(kc == 3))
                            mm(ps_q[:, s], ones, sq[:, s],
                               start=(kc == 0), stop=(kc == 3))
                    mean = lnt.tile([1, V], f32, tag="mean")
                    nc.scalar.mul(mean, ps_s, 1.0 / D)
                    msq = lnt.tile([1, V], f32, tag="msq")
                    nc.vector.tensor_mul(msq, mean, mean)
                    std = lnt.tile([1, V], f32, tag="std")
                    nc.vector.scalar_tensor_tensor(std, ps_q, 1.0 / D, msq,
                                                   ALU.mult, ALU.subtract)
                    nc.scalar.activation(std, std, ACTF.Sqrt, bias=epst)
                    sT = lnt.tile([128, 8], f32, tag="sT")
                    for c in range(8):
                        nc.sync.dma_start(out=sT[:, c:c + 1],
                                          in_=std[0:1,
                                                  c * 128:(c + 1) * 128])
                    rT = lnt.tile([128, 8], f32, tag="rT")
                    nc.vector.reciprocal(rT, sT)
                    r = lnt.tile([1, V], f32, tag="r")
                    for c in range(8):
                        nc.sync.dma_start(out=r[0:1,
                                              c * 128:(c + 1) * 128],
                                          in_=rT[:, c:c + 1])
                    mr = lnt.tile([1, V], f32, tag="mr")
                    nc.vector.tensor_mul(mr, mean, r)
                    rb = lnt.tile([128, V], f32, tag="rb")
                    nc.gpsimd.partition_broadcast(rb, r)
                    mrb = lnt.tile([128, V], f32, tag="mrb")
                    nc.gpsimd.partition_broadcast(mrb, mr)
                    for kc in range(4):
                        smd = ln_idx * 8 + kc
                        tmd = ln_idx * 8 + 4 + kc
                        u = lnt.tile([128, V], f32, tag="lnu", bufs=2,
                                     name="u")
                        nc.vector.tensor_mul(u, src_tiles[kc].bitcast(f32),
                                             rb)
                        u2 = lnt.tile([128, V], f32, tag="lnu2", bufs=2,
                                      name="u2")
                        nc.vector.scalar_tensor_tensor(
                            u2, mrb, -1.0, u, ALU.mult, ALU.add)
                        nc.vector.tensor_scalar(out[kc], u2,
                                                params[:, smd:smd + 1],
                                                params[:, tmd:tmd + 1],
                                                ALU.mult, ALU.add)
                return out

            # ---- attention-lifetime pool: qk 32K + vaug 16.25K + att 16K ----
            with tc.tile_pool(name="attlife", bufs=1) as attlife:
                qk = [attlife.tile([128, V], bf16, tag=f"qk{m}",
                                   name=f"qk{m}") for m in range(8)]
                vaug = [attlife.tile([128, 8, 65], bf16, tag=f"vaug{t}",
                                     name=f"vaug{t}") for t in range(8)]
                att = [attlife.tile([128, V], f32r, tag=f"att{kc}",
                                    name=f"att{kc}") for kc in range(4)]

                # h1 = AdaLN1(x); qk feature-major; v token-major
                with tc.tile_pool(name="h1pool", bufs=1) as h1pool:
                    h1 = adaln(xT_t, 0, h1pool, "h1")
                    with tc.tile_pool(name="qkvw", bufs=1) as qkvw, \
                            tc.tile_pool(name="qkvp", bufs=4,
                                         space="PSUM") as qkvp:
                        wqk_t = [qkvw.tile([128, 1024], f32r, tag=f"wqk{kc}",
                                           name="wqk_t") for kc in range(4)]
                        wv_t = [qkvw.tile([128, 512], f32r, tag=f"wv{kc}",
                                          name="wv_t") for kc in range(4)]
                        for kc in range(4):
                            nc.sync.dma_start(
                                out=wqk_t[kc],
                                in_=wqk[kc * 128:(kc + 1) * 128, :])
                            nc.sync.dma_start(
                                out=wv_t[kc],
                                in_=wv[kc * 128:(kc + 1) * 128, :])
                        for m in range(8):
                            for nh in range(2):
                                s = slice(nh * 512, nh * 512 + 512)
                                pp = qkvp.tile([128, 512], f32, tag="mmqk")
                                for kc in range(4):
                                    mm(pp,
                                       wqk_t[kc][:, m * 128:(m + 1) * 128],
                                       h1[kc][:, s], start=(kc == 0),
                                       stop=(kc == 3))
                                nc.any.tensor_copy(out=qk[m][:, s], in_=pp)
                        for t in range(8):
                            pp = qkvp.tile([128, 512], f32, tag="mmv")
                            for kc in range(4):
                                mm(pp, h1[kc][:, t * 128:(t + 1) * 128],
                                   wv_t[kc], start=(kc == 0), stop=(kc == 3))
                            nc.any.tensor_copy(
                                out=vaug[t][:, :, 0:64],
                                in_=pp[:].rearrange("p (h d) -> p h d", h=8))
                            nc.sync.dma_start(out=vaug[t][:, :, 64:65],
                                              in_=onesb[:].rearrange("p (h o) -> p h o", o=1))
                        if probe:
                            for kc in range(4):
                                nc.sync.dma_start(out=p_h1[kc * 128:(kc + 1) * 128, :], in_=h1[kc].bitcast(f32))
                            for m in range(8):
                                nc.sync.dma_start(out=p_qk[m], in_=qk[m].bitcast(f32))
                            for t2 in range(8):
                                nc.sync.dma_start(out=p_vaug[t2], in_=vaug[t2].bitcast(f32))

                # attention: S^T[j,i], biased softmax over j (partitions)
                with tc.tile_pool(name="attt", bufs=1) as attt, \
                        tc.tile_pool(name="attps", bufs=2,
                                     space="PSUM") as attps, \
                        tc.tile_pool(name="attpo", bufs=1,
                                     space="PSUM") as attpo:
                    masks = [[None] * 3 for _ in range(8)]
                    for jt in range(8):
                        eit = attt.tile([128, V], bf16, tag="eit",
                                        bufs=2, name="eit")
                        nc.sync.dma_start(out=eit,
                                          in_=eiT[jt * 128:(jt + 1) * 128, :])
                        for e in range(3):
                            mk = attt.tile([128, V], bf16,
                                           tag=f"mask{jt}_{e}", bufs=1,
                                           name="mk")
                            nc.vector.tensor_scalar(
                                mk, eit, float(e), None, ALU.is_equal)
                            masks[jt][e] = mk
                    for hg in range(4):
                        ops = [attpo.tile([65, V], f32, tag=f"ops{i}",
                                          bufs=1, name=f"ops{i}")
                               for i in range(2)]
                        for jt in range(8):
                            jsl = slice(jt * 128, jt * 128 + 128)
                            for hi in range(2):
                                h = hg * 2 + hi
                                kt = qk[4 + h // 2][
                                    (h % 2) * 64:(h % 2) * 64 + 64, jsl]
                                S = attps.tile([128, V], f32, tag="mms",
                                               name="S")
                                for nh in range(2):
                                    s = slice(nh * 512, nh * 512 + 512)
                                    qt = qk[h // 2][
                                        (h % 2) * 64:(h % 2) * 64 + 64, s]
                                    nc.tensor.matmul(S[:, s], kt, qt,
                                                     start=True, stop=True)
                                sm1 = attt.tile([128, V], bf16,
                                                tag="sm1", bufs=2,
                                                name="sm1")
                                nc.vector.tensor_scalar(
                                    sm1, masks[jt][1], cb[1][h], None,
                                    ALU.mult)
                                sm2 = attt.tile([128, V], bf16,
                                                tag="sm2", bufs=2,
                                                name="sm2")
                                nc.vector.tensor_scalar(
                                    sm2, masks[jt][2], cb[2][h], None,
                                    ALU.mult)
                                L1 = attt.tile([128, V], bf16,
                                               tag="logits1", bufs=2,
                                               name="L1")
                                nc.vector.scalar_tensor_tensor(
                                    L1, masks[jt][0], cb[0][h], S,
                                    ALU.mult, ALU.add)
                                L2 = attt.tile([128, V], bf16,
                                               tag="logits2", bufs=2,
                                               name="L2")
                                nc.gpsimd.tensor_add(L2, L1, sm1)
                                L3 = attt.tile([128, V], bf16,
                                               tag="logits3", bufs=2,
                                               name="L3")
                                nc.vector.tensor_add(L3, L2, sm2)
                                P = attt.tile([128, V], bf16, tag="probs",
                                              bufs=2, name="P")
                                nc.scalar.activation(P, L3, ACTF.Exp)
                                if probe and jt == 0 and hg == 0 and hi == 0:
                                    nc.sync.dma_start(out=p_L[:],
                                                      in_=L3[:, 0:512])
                                    nc.sync.dma_start(out=p_P[:],
                                                      in_=P[:, 0:512])
                                    nc.sync.dma_start(out=p_m0[:],
                                                      in_=masks[jt][0][:,
                                                                       0:512])
                                for nh in range(2):
                                    s = slice(nh * 512, nh * 512 + 512)
                                    nc.tensor.matmul(
                                        ops[hi][:, s], vaug[jt][:, h, :],
                                        P[:, s], start=(jt == 0),
                                        stop=(jt == 7))
                        # divide by the ones-row sums
                        if probe and hg == 0:
                            opsb = attt.tile([65, 512], f32, tag="opsb",
                                             name="opsb")
                            nc.any.tensor_copy(out=opsb,
                                               in_=ops[0][:, 0:512])
                            nc.sync.dma_start(out=p_ops[:], in_=opsb)
                        for hi in range(2):
                            h = hg * 2 + hi
                            ls = attt.tile([65, V], f32, tag="ls",
                                           bufs=2, name="ls")
                            nc.scalar.copy(ls[64:65, :], ops[hi][64:65, :])
                            lT = attt.tile([128, 8], f32, tag="lT",
                                           bufs=2, name="lT")
                            for c in range(8):
                                nc.sync.dma_start(
                                    out=lT[:, c:c + 1],
                                    in_=ls[64:65,
                                           c * 128:(c + 1) * 128])
                            rlT = attt.tile([128, 8], f32, tag="rlT",
                                            bufs=2, name="rlT")
                            nc.vector.reciprocal(rlT, lT)
                            rl_s = attt.tile([1, V], f32, tag="rls",
                                             bufs=2, name="rl_s")
                            for c in range(8):
                                nc.sync.dma_start(
                                    out=rl_s[0:1, c * 128:(c + 1) * 128],
                                    in_=rlT[:, c:c + 1])
                            rlb = attt.tile([64, V], f32, tag="rlb",
                                            bufs=2, name="rlb")
                            nc.gpsimd.partition_broadcast(rlb, rl_s)
                            nc.vector.tensor_mul(
                                att[h // 2][(h % 2) * 64:(h % 2) * 64 + 64,
                                            :],
                                ops[hi][0:64, :], rlb)

                # proj + residual (in place into xT_t)
                with tc.tile_pool(name="projw", bufs=1) as projw, \
                        tc.tile_pool(name="projp", bufs=4,
                                     space="PSUM") as projp:
                    wp_t = [projw.tile([128, 512], f32r, tag=f"wproj{kc}",
                                       name="wp_t") for kc in range(4)]
                    for kc in range(4):
                        nc.sync.dma_start(
                            out=wp_t[kc],
                            in_=wproj[kc * 128:(kc + 1) * 128, :])
                    for m in range(4):
                        for nh in range(2):
                            s = slice(nh * 512, nh * 512 + 512)
                            pp = projp.tile([128, 512], f32, tag="mmproj")
                            for kc in range(4):
                                mm(pp, wp_t[kc][:, m * 128:(m + 1) * 128],
                                   att[kc][:, s], start=(kc == 0),
                                   stop=(kc == 3))
                            nc.vector.scalar_tensor_tensor(
                                x2[m][:, s], pp, bp_t[:, m:m + 1],
                                xT_t[m][:, s].bitcast(f32), ALU.add,
                                ALU.add)
                    if probe:
                        for kc in range(4):
                            nc.sync.dma_start(out=p_att[kc * 128:(kc + 1) * 128, :], in_=att[kc].bitcast(f32))
                            nc.sync.dma_start(out=p_x2[kc * 128:(kc + 1) * 128, :], in_=x2[kc].bitcast(f32))

            # ---------- MLP branch (xT_t now holds x2) ----------
            with tc.tile_pool(name="mlplife", bufs=1) as mlplife:
                h2 = adaln(x2, 1, mlplife, "h2")
                if probe:
                    for kc in range(4):
                        nc.sync.dma_start(out=p_h2[kc * 128:(kc + 1) * 128, :], in_=h2[kc].bitcast(f32))
                with tc.tile_pool(name="mlpw", bufs=1) as mlpw, \
                        tc.tile_pool(name="mlpt", bufs=1) as mlpt, \
                        tc.tile_pool(name="mlpp", bufs=4,
                                     space="PSUM") as mlpp:
                    wm1_t = [mlpw.tile([128, 2048], f32r, tag=f"wm1{kc}",
                                       name="wm1_t") for kc in range(4)]
                    for kc in range(4):
                        nc.sync.dma_start(
                            out=wm1_t[kc],
                            in_=wm1[kc * 128:(kc + 1) * 128, :])
                    wm2_t = [mlpw.tile([128, 512], f32r, tag=f"wm2{kc}",
                                       name="wm2_t") for kc in range(16)]
                    for kc in range(16):
                        nc.sync.dma_start(
                            out=wm2_t[kc],
                            in_=wm2[kc * 128:(kc + 1) * 128, :])
                    for nh in range(2):
                        s = slice(nh * 512, nh * 512 + 512)
                        g = [mlpt.tile([128, 512], f32r, tag=f"g{m}",
                                       name=f"g{m}") for m in range(16)]
                        for m in range(16):
                            pp = mlpp.tile([128, 512], f32, tag="mmm1")
                            for kc in range(4):
                                mm(pp, wm1_t[kc][:, m * 128:(m + 1) * 128],
                                   h2[kc][:, s], start=(kc == 0),
                                   stop=(kc == 3))
                            nc.scalar.activation(g[m], pp, ACTF.Gelu,
                                                 bias=bm1_t[:, m:m + 1])
                        for m in range(4):
                            pp = mlpp.tile([128, 512], f32, tag="mmm2")
                            for kc in range(16):
                                mm(pp, wm2_t[kc][:, m * 128:(m + 1) * 128],
                                   g[kc], start=(kc == 0), stop=(kc == 15))
                            yt = mlpt.tile([128, 512], f32, tag="yt",
                                           bufs=2, name="yt")
                            nc.vector.scalar_tensor_tensor(
                                yt, pp, bm2_t[:, m:m + 1],
                                x2[m][:, s].bitcast(f32), ALU.add,
                                ALU.add)
                            nc.sync.dma_start(
                                out=yT[m * 128:(m + 1) * 128, s], in_=yt)

    nc.compile()
    return nc


def _make_in_maps(inputs):
    x = np.asarray(inputs["x"], dtype=np.float32)
    cond = np.asarray(inputs["cond"], dtype=np.float32)
    ei = np.asarray(inputs["edge_index"])
    w_qkv = np.asarray(inputs["w_qkv"], dtype=np.float32)

    scale = 1.0 / np.sqrt(HD)
    wqk = w_qkv[:, :2 * D].copy()
    wqk[:, :D] *= scale
    wv = np.ascontiguousarray(w_qkv[:, 2 * D:])
    wada = np.concatenate([inputs["w_ada1"], inputs["w_ada2"]],
                          axis=1).astype(np.float32)
    bada = np.concatenate([inputs["b_ada1"], inputs["b_ada2"]]).astype(
        np.float32).copy()
    bada[:D] += 1.0          # fold the (1 + scale) into ada1 scale bias
    bada[2 * D:3 * D] += 1.0  # and ada2 scale bias

    shared = {
        "onesc": np.ones((128, 8), dtype=np.float32),
        "onesb": np.ones((128, 8), dtype=ml_dtypes.bfloat16),
        "wqk": np.ascontiguousarray(wqk),
        "wv": wv,
        "wada": np.ascontiguousarray(wada),
        "bada": np.ascontiguousarray(bada.reshape(16, 128)),
        "wproj": np.ascontiguousarray(inputs["w_proj"].astype(np.float32)),
        "bproj": np.ascontiguousarray(
            inputs["b_proj"].astype(np.float32).reshape(4, 128)),
        "wm1": np.ascontiguousarray(inputs["w_mlp1"].astype(np.float32)),
        "bm1": np.ascontiguousarray(
            inputs["b_mlp1"].astype(np.float32).reshape(16, 128)),
        "wm2": np.ascontiguousarray(inputs["w_mlp2"].astype(np.float32)),
        "bm2": np.ascontiguousarray(
            inputs["b_mlp2"].astype(np.float32).reshape(4, 128)),
    }
    in_maps = []
    for b in range(B):
        in_maps.append(dict(
            shared,
            xT=np.ascontiguousarray(x[b].T),
            eiT=np.ascontiguousarray(ei[b].T.astype(ml_dtypes.bfloat16)),
            condc=np.ascontiguousarray(cond[b].reshape(4, 128)),
        ))
    return in_maps


def kernel(**inputs):
    from concourse.bass_utils import run_bass_kernel_spmd

    et = np.asarray(inputs["edge_table"], dtype=np.float32)
    cb = [[float(et[e, h] - et[3, h]) for h in range(H)] for e in range(3)]

    key = (et.tobytes(),)
    if key not in _cache:
        _cache[key] = _build_program(cb)
    nc = _cache[key]

    in_maps = _make_in_maps(inputs)
    res = run_bass_kernel_spmd(nc, in_maps, core_ids=list(range(NCORES)))
    out = np.stack([np.ascontiguousarray(res.results[b]["yT"].T)
                    for b in range(B)])
    return out.astype(np.float32)



# revision 23
# speedup vs baseline: 1.9053x; 1.9053x over previous
"""MeshMeanFlowNet block on 8 Trainium2 NeuronCores.

Sharding: data-parallel over B (one batch element per core), no collectives.

Key design points vs the naive formulation:
- All activations feature-major ([feature, token]); attention softmax in the
  transposed layout S^T[j, i] (key j on partitions) so the PV matmul consumes
  probabilities directly as the moving operand and the softmax denominator
  falls out of a ones-row appended to V.
- The per-edge-type/per-head bias is applied MULTIPLICATIVELY after exp:
  P = exp(S) * g, with g[h][j,i] = exp(edge_table[edge[i,j], h]) precomputed
  on the host and streamed from HBM as bf16 tiles. This removes every
  mask/select elementwise op from the device inner loop (1 Act exp + 1 DVE
  bf16 multiply per (key-tile, head) pair).
- LayerNorm affine is applied as h = x*R1 + Dt where R1 = s1 (x) r and
  Dt = t1 (x) 1 - s1 (x) (m*r) are built by rank-1/rank-2 PE matmuls into
  PSUM (2 DVE passes per 128-feature tile, no gpsimd broadcasts, no
  column-transpose DMA storms).
- Softmax normalization: Z rows extracted from PSUM, packed into an [8, V]
  tile via tiny SBUF->SBUF DMAs, one reciprocal, then per-feature-tile
  band-broadcast via contraction-2 PE matmuls; one in-place DVE multiply
  per attention-output tile.
- Weights are shipped bf16 (halves weight HBM traffic); f32r only where
  fp32-ish accumulation inputs matter (x, LN sums).
"""

import sys

sys.path.insert(0, "/opt/trn_rl_repo")

import ml_dtypes
import numpy as np

B, V, D, H = 8, 1024, 512, 8
HD = D // H  # 64
NCORES = 8

_cache = {}


def _build_program(sim_mode=False):
    import contextlib

    import concourse.bacc as bacc
    import concourse.tile as tile
    from concourse import mybir

    f32 = mybir.dt.float32
    f32r = mybir.dt.float32r
    bf16 = mybir.dt.bfloat16
    ALU = mybir.AluOpType
    ACTF = mybir.ActivationFunctionType

    nc = bacc.Bacc("TRN2", target_bir_lowering=False, debug=False,
                   num_devices=NCORES)

    # ---- DRAM I/O (per-core shard, host pre-laid-out) ----
    xT = nc.dram_tensor("xT", [D, V], f32r, kind="ExternalInput")
    condc = nc.dram_tensor("condc", [4, 128], f32, kind="ExternalInput")
    gbias = nc.dram_tensor("gbias", [H * V, V], bf16, kind="ExternalInput")
    wqk = nc.dram_tensor("wqk", [D, 1024], bf16, kind="ExternalInput")
    wv = nc.dram_tensor("wv", [D, 512], bf16, kind="ExternalInput")
    wada = nc.dram_tensor("wada", [D, 2048], bf16, kind="ExternalInput")
    badar = nc.dram_tensor("badar", [1, 2048], f32, kind="ExternalInput")
    wproj = nc.dram_tensor("wproj", [D, D], bf16, kind="ExternalInput")
    bproj = nc.dram_tensor("bproj", [4, 128], f32, kind="ExternalInput")
    wm1 = nc.dram_tensor("wm1", [D, 2048], bf16, kind="ExternalInput")
    bm1 = nc.dram_tensor("bm1", [16, 128], f32, kind="ExternalInput")
    wm2 = nc.dram_tensor("wm2", [2048, D], bf16, kind="ExternalInput")
    bm2 = nc.dram_tensor("bm2", [4, 128], f32, kind="ExternalInput")
    bandd = nc.dram_tensor("bandd", [2, 128], bf16, kind="ExternalInput")
    onesd = nc.dram_tensor("onesd", [128, 1], f32r, kind="ExternalInput")
    onesrowd = nc.dram_tensor("onesrowd", [1, 1024], f32r,
                              kind="ExternalInput")
    yT = nc.dram_tensor("yT", [D, V], f32, kind="ExternalOutput")

    def mm(out, lhsT, rhs, **kw):
        nc.tensor.matmul(out, lhsT, rhs, **kw)

    with tile.TileContext(nc) as tc:
        with contextlib.ExitStack() as ctx:
            persist = ctx.enter_context(tc.tile_pool(name="persist", bufs=1))

            ones = persist.tile([128, 1], f32r, tag="ones")
            nc.sync.dma_start(out=ones, in_=onesd[:])
            onesrow = persist.tile([1, V], f32r, tag="onesrow")
            nc.sync.dma_start(out=onesrow, in_=onesrowd[:])
            epst = persist.tile([1, 1], f32, tag="eps")
            nc.vector.memset(epst, 1e-5)
            # band indicator for Z-broadcast: row0 -> out partitions 0-63,
            # row1 -> 64-127
            bandm = persist.tile([2, 128], bf16, tag="bandm")
            nc.sync.dma_start(out=bandm, in_=bandd[:])

            # x (feature-major, also becomes x2 in place after the residual)
            xT_t = [persist.tile([128, V], f32r, tag=f"xT{kc}",
                                 name=f"xT_t{kc}") for kc in range(4)]
            for kc in range(4):
                nc.sync.dma_start(out=xT_t[kc],
                                  in_=xT[kc * 128:(kc + 1) * 128, :])

            bp_t = persist.tile([128, 4], f32, tag="bproj")
            nc.sync.dma_start(out=bp_t, in_=bproj[:].rearrange("c p -> p c"))
            bm1_t = persist.tile([128, 16], f32, tag="bm1")
            nc.sync.dma_start(out=bm1_t, in_=bm1[:].rearrange("c p -> p c"))
            bm2_t = persist.tile([128, 4], f32, tag="bm2")
            nc.sync.dma_start(out=bm2_t, in_=bm2[:].rearrange("c p -> p c"))

            # row-form AdaLN params: [1, 2048] =
            # [ada1: scale(512) shift(512) | ada2: scale shift] (+1 folded
            # into scale on host)
            rows_params = persist.tile([1, 2048], f32r, tag="rparams")

            # ---------- AdaLN parameter path ----------
            with tc.tile_pool(name="adaw", bufs=1) as adaw, \
                    tc.tile_pool(name="adap", bufs=1, space="PSUM") as adap:
                condt = adaw.tile([128, 5], f32, tag="cond")
                nc.sync.dma_start(out=condt[:, 0:4],
                                  in_=condc[:].rearrange("c p -> p c"))
                nc.vector.memset(condt[:, 4:5], 0.0)
                scond = adaw.tile([128, 5], bf16, tag="scond")
                if sim_mode:
                    sig = adaw.tile([128, 5], f32, tag="sig")
                    nc.scalar.activation(sig, condt, ACTF.Sigmoid)
                    nc.vector.tensor_mul(scond, condt, sig)
                else:
                    nc.scalar.activation(scond, condt, ACTF.Silu)
                badar_t = adaw.tile([1, 2048], f32, tag="badar")
                nc.sync.dma_start(out=badar_t, in_=badar[:])
                wada_t = [adaw.tile([128, 2048], bf16, tag=f"wada{kc}",
                                    name="wada_t") for kc in range(4)]
                for kc in range(4):
                    nc.sync.dma_start(out=wada_t[kc],
                                      in_=wada[kc * 128:(kc + 1) * 128, :])
                pp = adap.tile([2, 2048], f32, tag="pada")
                for oc in range(4):
                    s = slice(oc * 512, oc * 512 + 512)
                    for kc in range(4):
                        mm(pp[:, s], scond[:, kc:kc + 2], wada_t[kc][:, s],
                           start=(kc == 0), stop=(kc == 3))
                nc.vector.tensor_add(rows_params, pp[0:1, :], badar_t)

            def adaln(src_tiles, ln_idx, dst_pool, out_tag):
                """Feature-axis LayerNorm + adaptive affine; returns 4
                feature-major bf16 tiles."""
                out = [dst_pool.tile([128, V], bf16, tag=f"{out_tag}{kc}",
                                     name=f"ln_{out_tag}{kc}")
                       for kc in range(4)]
                base = ln_idx * 1024
                with tc.tile_pool(name="lnt", bufs=1) as lnt:
                    with tc.tile_pool(name="lnp", bufs=1,
                                      space="PSUM") as lnp:
                        ps_s = lnp.tile([1, V], f32, tag="lnsum")
                        ps_q = lnp.tile([1, V], f32, tag="lnsqsum")
                        for kc in range(4):
                            sq = lnt.tile([128, V], f32r, tag="lnsq", bufs=2,
                                          name="sq")
                            nc.gpsimd.tensor_mul(sq,
                                                 src_tiles[kc].bitcast(f32),
                                                 src_tiles[kc].bitcast(f32))
                            for nh in range(2):
                                s = slice(nh * 512, nh * 512 + 512)
                                mm(ps_s[:, s], ones, src_tiles[kc][:, s],
                                   start=(kc == 0), stop=(kc == 3))
                                mm(ps_q[:, s], ones, sq[:, s],
                                   start=(kc == 0), stop=(kc == 3))
                        m_row = lnt.tile([1, V], f32, tag="mrow")
                        nc.scalar.mul(m_row, ps_s, 1.0 / D)
                        msq = lnt.tile([1, V], f32, tag="msq")
                        nc.scalar.square(msq, m_row)
                        varr = lnt.tile([1, V], f32, tag="varr")
                        nc.vector.scalar_tensor_tensor(
                            varr, ps_q, 1.0 / D, msq, ALU.mult, ALU.subtract)
                    stdr = lnt.tile([1, V], f32, tag="stdr")
                    nc.scalar.activation(stdr, varr, ACTF.Sqrt, bias=epst)
                    r_row = lnt.tile([1, V], f32r, tag="rrow")
                    with nc.allow_low_precision(
                            reason="1/std as f32r matmul operand"):
                        nc.vector.reciprocal(r_row, stdr)
                    mrneg = lnt.tile([1, V], f32r, tag="mrneg")
                    nc.vector.scalar_tensor_tensor(
                        mrneg, m_row, -1.0, r_row.bitcast(f32),
                        ALU.mult, ALU.mult)
                    with tc.tile_pool(name="lnbp", bufs=2,
                                      space="PSUM") as lnbp:
                        for kc in range(4):
                            s1r = rows_params[0:1, base + kc * 128:
                                              base + kc * 128 + 128]
                            t1r = rows_params[0:1, base + 512 + kc * 128:
                                              base + 512 + kc * 128 + 128]
                            R1 = lnbp.tile([128, V], f32, tag="R1",
                                           name="R1")
                            Dt = lnbp.tile([128, V], f32, tag="Dt",
                                           name="Dt")
                            for nh in range(2):
                                s = slice(nh * 512, nh * 512 + 512)
                                mm(R1[:, s], s1r,
                                   r_row[:, s], start=True, stop=True)
                                mm(Dt[:, s], t1r,
                                   onesrow[:, s], start=True, stop=False)
                                mm(Dt[:, s], s1r,
                                   mrneg[:, s], start=False, stop=True)
                            u = lnt.tile([128, V], f32, tag="lnu", bufs=2,
                                         name="u")
                            nc.vector.tensor_mul(u, src_tiles[kc].bitcast(f32),
                                                 R1)
                            nc.vector.tensor_add(out[kc], u, Dt)
                return out

            # proj+mlp weights: pool outlives the attention scope; DMAs are
            # issued inside the attention loop so they stream during it
            mlpw = ctx.enter_context(tc.tile_pool(name="mlpw", bufs=1))
            wp_t = [mlpw.tile([128, 512], bf16, tag=f"wproj{kc}",
                              name="wp_t") for kc in range(4)]
            wm1_t = [mlpw.tile([128, 2048], bf16, tag=f"wm1{kc}",
                               name="wm1_t") for kc in range(4)]
            wm2_t = [mlpw.tile([128, 512], bf16, tag=f"wm2{kc}",
                               name="wm2_t") for kc in range(16)]

            # qk tiles (feature-major q then k), token-major v (+ones row)
            with tc.tile_pool(name="attlife", bufs=1) as attlife:
                qk = [attlife.tile([128, V], bf16, tag=f"qk{m}",
                                   name=f"qk{m}") for m in range(8)]
                vaug = [attlife.tile([128, 8, 65], bf16, tag=f"vaug{t}",
                                     name=f"vaug{t}") for t in range(8)]
                att = [attlife.tile([128, V], bf16, tag=f"att{kc}",
                                    name=f"att{kc}") for kc in range(4)]
                # Z rows packed [parity, kc*V + i]: head h -> row h%2,
                # free slice (h//2)*V
                zcat = attlife.tile([2, 4 * V], bf16, tag="zcat")

                # h1 = AdaLN1(x); qk feature-major; v token-major
                with tc.tile_pool(name="h1pool", bufs=1) as h1pool:
                    h1 = adaln(xT_t, 0, h1pool, "h1")
                    with tc.tile_pool(name="qkvw", bufs=1) as qkvw, \
                            tc.tile_pool(name="qkvp", bufs=4,
                                         space="PSUM") as qkvp:
                        wqk_t = [qkvw.tile([128, 1024], bf16, tag=f"wqk{kc}",
                                           name="wqk_t") for kc in range(4)]
                        wv_t = [qkvw.tile([128, 512], bf16, tag=f"wv{kc}",
                                          name="wv_t") for kc in range(4)]
                        for kc in range(4):
                            nc.sync.dma_start(
                                out=wqk_t[kc],
                                in_=wqk[kc * 128:(kc + 1) * 128, :])
                            nc.sync.dma_start(
                                out=wv_t[kc],
                                in_=wv[kc * 128:(kc + 1) * 128, :])
                        for m in range(8):
                            for nh in range(2):
                                s = slice(nh * 512, nh * 512 + 512)
                                pp = qkvp.tile([128, 512], f32, tag="mmqk")
                                for kc in range(4):
                                    mm(pp,
                                       wqk_t[kc][:, m * 128:(m + 1) * 128],
                                       h1[kc][:, s], start=(kc == 0),
                                       stop=(kc == 3))
                                nc.vector.tensor_copy(out=qk[m][:, s], in_=pp)
                        for t in range(8):
                            pp = qkvp.tile([128, 512], f32, tag="mmv")
                            for kc in range(4):
                                mm(pp, h1[kc][:, t * 128:(t + 1) * 128],
                                   wv_t[kc], start=(kc == 0), stop=(kc == 3))
                            nc.vector.tensor_copy(
                                out=vaug[t][:, :, 0:64],
                                in_=pp[:].rearrange("p (h d) -> p h d", h=8))
                            nc.gpsimd.memset(vaug[t][:, :, 64:65], 1.0)
                        # proj + mlp1 weights stream while qkv/attention
                        # compute; wm2 is issued at the end of the g stream
                        for kc in range(4):
                            nc.sync.dma_start(
                                out=wp_t[kc],
                                in_=wproj[kc * 128:(kc + 1) * 128, :])
                        for kc in range(4):
                            nc.sync.dma_start(
                                out=wm1_t[kc],
                                in_=wm1[kc * 128:(kc + 1) * 128, :])

                # attention: S^T[j,i]; P = exp(S) * g; softmax denom from
                # the vaug ones-row, normalized after the loop.
                with tc.tile_pool(name="attt", bufs=1) as attt:
                  with tc.tile_pool(name="attps", bufs=2,
                                    space="PSUM") as attps, \
                        tc.tile_pool(name="attpo", bufs=1,
                                     space="PSUM") as attpo:
                    first = True
                    for hg in range(4):
                        ops = [attpo.tile([65, V], f32, tag=f"ops{i}",
                                          bufs=1, name=f"ops{i}")
                               for i in range(2)]
                        for jt in range(8):
                            jsl = slice(jt * 128, jt * 128 + 128)
                            for hi in range(2):
                                h = hg * 2 + hi
                                g_t = attt.tile([128, V], bf16, tag="gt",
                                                bufs=10, name="g_t")
                                nc.sync.dma_start(
                                    out=g_t,
                                    in_=gbias[(h * 8 + jt) * 128:
                                              (h * 8 + jt) * 128 + 128, :])
                                if hg == 3 and jt == 0 and hi == 0:
                                    for kc in range(16):
                                        nc.sync.dma_start(
                                            out=wm2_t[kc],
                                            in_=wm2[kc * 128:
                                                    (kc + 1) * 128, :])
                                kt = qk[4 + h // 2][
                                    (h % 2) * 64:(h % 2) * 64 + 64, jsl]
                                S = attps.tile([128, V], f32, tag="mms",
                                               name="S")
                                for nh in range(2):
                                    s = slice(nh * 512, nh * 512 + 512)
                                    qt = qk[h // 2][
                                        (h % 2) * 64:(h % 2) * 64 + 64, s]
                                    mm(S[:, s], kt, qt, start=True, stop=True)
                                P0 = attt.tile([128, V], bf16, tag="P0",
                                               bufs=3, name="P0")
                                nc.scalar.activation(P0, S, ACTF.Exp)
                                P = attt.tile([128, V], bf16, tag="P",
                                              bufs=3, name="P")
                                nc.vector.tensor_mul(P, P0, g_t)
                                for nh in range(2):
                                    s = slice(nh * 512, nh * 512 + 512)
                                    mm(ops[hi][:, s], vaug[jt][:, h, :],
                                       P[:, s], start=(jt == 0),
                                       stop=(jt == 7))
                        for hi in range(2):
                            h = hg * 2 + hi
                            ztmp = attt.tile([65, V], bf16, tag="ztmp",
                                             bufs=2, name="ztmp")
                            nc.vector.tensor_copy(out=ztmp[64:65, :],
                                                  in_=ops[hi][64:65, :])
                            nc.sync.dma_start(
                                out=zcat[hi:hi + 1,
                                         hg * V:hg * V + V],
                                in_=ztmp[64:65, :])
                            nc.vector.tensor_copy(
                                out=att[h // 2][(h % 2) * 64:
                                                (h % 2) * 64 + 64, :],
                                in_=ops[hi][0:64, :])

                  # normalize: att[kc] rows 0-63 = head 2kc, 64-127 = 2kc+1
                  if True:
                    rinv = attt.tile([2, 4 * V], bf16, tag="rinv")
                    with nc.allow_low_precision(
                            reason="softmax denom reciprocal in bf16"):
                        nc.vector.reciprocal(rinv, zcat)
                    with tc.tile_pool(name="zbp", bufs=2,
                                      space="PSUM") as zbp:
                        for kc in range(4):
                            zb = zbp.tile([128, V], f32, tag="zb", name="zb")
                            for nh in range(2):
                                s = slice(nh * 512, nh * 512 + 512)
                                mm(zb[:, s], bandm,
                                   rinv[0:2, kc * V + nh * 512:
                                        kc * V + nh * 512 + 512],
                                   start=True, stop=True)
                            nc.vector.tensor_mul(att[kc], att[kc], zb)

                # proj + residual (in place into xT_t -> x2)
                with tc.tile_pool(name="projp", bufs=4,
                                  space="PSUM") as projp:
                    for m in range(4):
                        for nh in range(2):
                            s = slice(nh * 512, nh * 512 + 512)
                            pp = projp.tile([128, 512], f32, tag="mmproj")
                            for kc in range(4):
                                mm(pp, wp_t[kc][:, m * 128:(m + 1) * 128],
                                   att[kc][:, s], start=(kc == 0),
                                   stop=(kc == 3))
                            nc.vector.scalar_tensor_tensor(
                                xT_t[m][:, s], pp,
                                bp_t[:, m:m + 1],
                                xT_t[m][:, s].bitcast(f32), ALU.add,
                                ALU.add)

            # ---------- MLP branch (xT_t now holds x2) ----------
            with tc.tile_pool(name="mlplife", bufs=1) as mlplife:
                h2 = adaln(xT_t, 1, mlplife, "h2")
                with tc.tile_pool(name="mlpt", bufs=1) as mlpt, \
                        tc.tile_pool(name="mlpp", bufs=4,
                                     space="PSUM") as mlpp:
                    for nh in range(2):
                        s = slice(nh * 512, nh * 512 + 512)
                        gm = [mlpt.tile([128, 512], bf16, tag=f"gm{m}",
                                        name=f"gm{m}") for m in range(16)]
                        for m in range(16):
                            pp = mlpp.tile([128, 512], f32, tag="mmm1")
                            for kc in range(4):
                                mm(pp, wm1_t[kc][:, m * 128:(m + 1) * 128],
                                   h2[kc][:, s], start=(kc == 0),
                                   stop=(kc == 3))
                            if sim_mode:
                                sig = mlpt.tile([128, 512], f32, tag="gsig",
                                                bufs=2, name="gsig")
                                nc.scalar.activation(sig, pp, ACTF.Sigmoid,
                                                     scale=1.702)
                                nc.vector.tensor_mul(gm[m], pp, sig)
                            else:
                                nc.scalar.activation(gm[m], pp, ACTF.Gelu,
                                                     bias=bm1_t[:, m:m + 1])
                        for m in range(4):
                            pp = mlpp.tile([128, 512], f32, tag="mmm2")
                            for kc in range(16):
                                mm(pp, wm2_t[kc][:, m * 128:(m + 1) * 128],
                                   gm[kc], start=(kc == 0), stop=(kc == 15))
                            yt = mlpt.tile([128, 512], f32, tag="yt",
                                           bufs=2, name="yt")
                            nc.vector.scalar_tensor_tensor(
                                yt, pp, bm2_t[:, m:m + 1],
                                xT_t[m][:, s].bitcast(f32), ALU.add,
                                ALU.add)
                            nc.sync.dma_start(
                                out=yT[m * 128:(m + 1) * 128, s], in_=yt)

    nc.compile()
    return nc


def _make_in_maps(inputs):
    bf = ml_dtypes.bfloat16
    x = np.asarray(inputs["x"], dtype=np.float32)
    cond = np.asarray(inputs["cond"], dtype=np.float32)
    ei = np.asarray(inputs["edge_index"])
    w_qkv = np.asarray(inputs["w_qkv"], dtype=np.float32)
    et = np.asarray(inputs["edge_table"], dtype=np.float32)

    scale = 1.0 / np.sqrt(HD)
    wqk = w_qkv[:, :2 * D].copy()
    wqk[:, :D] *= scale
    wv = np.ascontiguousarray(w_qkv[:, 2 * D:])
    wada = np.concatenate([inputs["w_ada1"], inputs["w_ada2"]],
                          axis=1).astype(np.float32)
    badar = np.concatenate([inputs["b_ada1"], inputs["b_ada2"]]).astype(
        np.float32).copy()
    badar[:D] += 1.0          # fold the (1 + scale) into ada1 scale bias
    badar[2 * D:3 * D] += 1.0  # and ada2 scale bias

    etT = np.exp(et).T.astype(np.float32)  # [H, 4]

    shared = {
        "wqk": np.ascontiguousarray(wqk.astype(bf)),
        "wv": wv.astype(bf),
        "wada": np.ascontiguousarray(wada.astype(bf)),
        "badar": np.ascontiguousarray(badar.reshape(1, 2 * 2 * D)),
        "wproj": np.ascontiguousarray(
            inputs["w_proj"].astype(np.float32).astype(bf)),
        "bproj": np.ascontiguousarray(
            inputs["b_proj"].astype(np.float32).reshape(4, 128)),
        "wm1": np.ascontiguousarray(
            inputs["w_mlp1"].astype(np.float32).astype(bf)),
        "bm1": np.ascontiguousarray(
            inputs["b_mlp1"].astype(np.float32).reshape(16, 128)),
        "wm2": np.ascontiguousarray(
            inputs["w_mlp2"].astype(np.float32).astype(bf)),
        "bm2": np.ascontiguousarray(
            inputs["b_mlp2"].astype(np.float32).reshape(4, 128)),
        "onesd": np.ones((128, 1), dtype=np.float32),
        "onesrowd": np.ones((1, 1024), dtype=np.float32),
        "bandd": np.ascontiguousarray(np.concatenate([
            np.concatenate([np.ones(64), np.zeros(64)]),
            np.concatenate([np.zeros(64), np.ones(64)]),
        ]).reshape(2, 128).astype(bf)),
    }
    in_maps = []
    for b in range(B):
        # g[h, j, i] = exp(et[ei[i, j], h])
        g = etT[:, ei[b]]                      # [H, i, j]
        g = np.ascontiguousarray(g.transpose(0, 2, 1))  # [H, j, i]
        in_maps.append(dict(
            shared,
            xT=np.ascontiguousarray(x[b].T),
            condc=np.ascontiguousarray(cond[b].reshape(4, 128)),
            gbias=g.reshape(H * V, V).astype(bf),
        ))
    return in_maps


def kernel(**inputs):
    from concourse.bass_utils import run_bass_kernel_spmd

    if "prog" not in _cache:
        _cache["prog"] = _build_program()
    nc = _cache["prog"]

    in_maps = _make_in_maps(inputs)
    res = run_bass_kernel_spmd(nc, in_maps, core_ids=list(range(NCORES)))
    out = np.stack([np.ascontiguousarray(res.results[b]["yT"].T)
                    for b in range(B)])
    return out.astype(np.float32)


# revision 27
# speedup vs baseline: 2.2566x; 1.1844x over previous
"""MeshMeanFlowNet block on 8 Trainium2 NeuronCores.

Sharding: data-parallel over B (one batch element per core), no collectives.

Key design points vs the naive formulation:
- All activations feature-major ([feature, token]); attention softmax in the
  transposed layout S^T[j, i] (key j on partitions) so the PV matmul consumes
  probabilities directly as the moving operand and the softmax denominator
  falls out of a ones-row appended to V.
- The per-edge-type/per-head bias is applied MULTIPLICATIVELY after exp:
  P = exp(S) * g, with g[h][j,i] = exp(edge_table[edge[i,j], h]) precomputed
  on the host and streamed from HBM as bf16 tiles. This removes every
  mask/select elementwise op from the device inner loop (1 Act exp + 1 DVE
  bf16 multiply per (key-tile, head) pair).
- LayerNorm affine is applied as h = x*R1 + Dt where R1 = s1 (x) r and
  Dt = t1 (x) 1 - s1 (x) (m*r) are built by rank-1/rank-2 PE matmuls into
  PSUM (2 DVE passes per 128-feature tile, no gpsimd broadcasts, no
  column-transpose DMA storms).
- Softmax normalization: Z rows extracted from PSUM, packed into an [8, V]
  tile via tiny SBUF->SBUF DMAs, one reciprocal, then per-feature-tile
  band-broadcast via contraction-2 PE matmuls; one in-place DVE multiply
  per attention-output tile.
- Weights are shipped bf16 (halves weight HBM traffic); f32r only where
  fp32-ish accumulation inputs matter (x, LN sums).
"""

import sys

sys.path.insert(0, "/opt/trn_rl_repo")

import ml_dtypes
import numpy as np

B, V, D, H = 8, 1024, 512, 8
HD = D // H  # 64
NCORES = 8

_cache = {}


def _build_program(sim_mode=False):
    import contextlib

    import concourse.bacc as bacc
    import concourse.tile as tile
    from concourse import mybir

    f32 = mybir.dt.float32
    f32r = mybir.dt.float32r
    bf16 = mybir.dt.bfloat16
    ALU = mybir.AluOpType
    ACTF = mybir.ActivationFunctionType

    nc = bacc.Bacc("TRN2", target_bir_lowering=False, debug=False,
                   num_devices=NCORES)

    # ---- DRAM I/O (per-core shard, host pre-laid-out) ----
    xT = nc.dram_tensor("xT", [D, V], f32r, kind="ExternalInput")
    condc = nc.dram_tensor("condc", [4, 128], f32, kind="ExternalInput")
    gbias = nc.dram_tensor("gbias", [H * V, V], bf16, kind="ExternalInput")
    wqk = nc.dram_tensor("wqk", [D, 1024], bf16, kind="ExternalInput")
    wv = nc.dram_tensor("wv", [D, 512], bf16, kind="ExternalInput")
    wada = nc.dram_tensor("wada", [D, 2048], bf16, kind="ExternalInput")
    badar = nc.dram_tensor("badar", [1, 2048], f32, kind="ExternalInput")
    wproj = nc.dram_tensor("wproj", [D, D], bf16, kind="ExternalInput")
    bproj = nc.dram_tensor("bproj", [4, 128], f32, kind="ExternalInput")
    wm1 = nc.dram_tensor("wm1", [D, 2048], bf16, kind="ExternalInput")
    bm1 = nc.dram_tensor("bm1", [16, 128], f32, kind="ExternalInput")
    wm2 = nc.dram_tensor("wm2", [2048, D], bf16, kind="ExternalInput")
    bm2 = nc.dram_tensor("bm2", [4, 128], f32, kind="ExternalInput")
    bandd = nc.dram_tensor("bandd", [2, 128], bf16, kind="ExternalInput")
    onesd = nc.dram_tensor("onesd", [128, 1], f32r, kind="ExternalInput")
    onesrowd = nc.dram_tensor("onesrowd", [1, 1024], f32r,
                              kind="ExternalInput")
    yT = nc.dram_tensor("yT", [D, V], f32, kind="ExternalOutput")

    def mm(out, lhsT, rhs, **kw):
        nc.tensor.matmul(out, lhsT, rhs, **kw)

    with tile.TileContext(nc) as tc:
        with contextlib.ExitStack() as ctx:
            persist = ctx.enter_context(tc.tile_pool(name="persist", bufs=1))

            ones = persist.tile([128, 1], f32r, tag="ones")
            nc.sync.dma_start(out=ones, in_=onesd[:])
            onesrow = persist.tile([1, V], f32r, tag="onesrow")
            nc.sync.dma_start(out=onesrow, in_=onesrowd[:])
            epst = persist.tile([1, 1], f32, tag="eps")
            nc.vector.memset(epst, 1e-5)
            # band indicator for Z-broadcast: row0 -> out partitions 0-63,
            # row1 -> 64-127
            bandm = persist.tile([2, 128], bf16, tag="bandm")
            nc.sync.dma_start(out=bandm, in_=bandd[:])

            # x (feature-major, also becomes x2 in place after the residual)
            xT_t = [persist.tile([128, V], f32r, tag=f"xT{kc}",
                                 name=f"xT_t{kc}") for kc in range(4)]
            for kc in range(4):
                nc.sync.dma_start(out=xT_t[kc],
                                  in_=xT[kc * 128:(kc + 1) * 128, :])

            bp_t = persist.tile([128, 4], f32, tag="bproj")
            nc.sync.dma_start(out=bp_t, in_=bproj[:].rearrange("c p -> p c"))
            bm1_t = persist.tile([128, 16], f32, tag="bm1")
            nc.sync.dma_start(out=bm1_t, in_=bm1[:].rearrange("c p -> p c"))
            bm2_t = persist.tile([128, 4], f32, tag="bm2")
            nc.sync.dma_start(out=bm2_t, in_=bm2[:].rearrange("c p -> p c"))

            # row-form AdaLN params: [1, 2048] =
            # [ada1: scale(512) shift(512) | ada2: scale shift] (+1 folded
            # into scale on host)
            rows_params = persist.tile([1, 2048], f32r, tag="rparams")

            # ---------- AdaLN parameter path ----------
            with tc.tile_pool(name="adaw", bufs=1) as adaw, \
                    tc.tile_pool(name="adap", bufs=1, space="PSUM") as adap:
                condt = adaw.tile([128, 5], f32, tag="cond")
                nc.sync.dma_start(out=condt[:, 0:4],
                                  in_=condc[:].rearrange("c p -> p c"))
                nc.vector.memset(condt[:, 4:5], 0.0)
                scond = adaw.tile([128, 5], bf16, tag="scond")
                if sim_mode:
                    sig = adaw.tile([128, 5], f32, tag="sig")
                    nc.scalar.activation(sig, condt, ACTF.Sigmoid)
                    nc.vector.tensor_mul(scond, condt, sig)
                else:
                    nc.scalar.activation(scond, condt, ACTF.Silu)
                badar_t = adaw.tile([1, 2048], f32, tag="badar")
                nc.sync.dma_start(out=badar_t, in_=badar[:])
                wada_t = [adaw.tile([128, 2048], bf16, tag=f"wada{kc}",
                                    name="wada_t") for kc in range(4)]
                for kc in range(4):
                    nc.sync.dma_start(out=wada_t[kc],
                                      in_=wada[kc * 128:(kc + 1) * 128, :])
                pp = adap.tile([2, 2048], f32, tag="pada")
                for oc in range(4):
                    s = slice(oc * 512, oc * 512 + 512)
                    for kc in range(4):
                        mm(pp[:, s], scond[:, kc:kc + 2], wada_t[kc][:, s],
                           start=(kc == 0), stop=(kc == 3))
                nc.vector.tensor_add(rows_params, pp[0:1, :], badar_t)

            def adaln(src_tiles, ln_idx, dst_pool, out_tag):
                """Feature-axis LayerNorm + adaptive affine; returns 4
                feature-major bf16 tiles."""
                out = [dst_pool.tile([128, V], bf16, tag=f"{out_tag}{kc}",
                                     name=f"ln_{out_tag}{kc}")
                       for kc in range(4)]
                base = ln_idx * 1024
                with tc.tile_pool(name="lnt", bufs=1) as lnt:
                    with tc.tile_pool(name="lnp", bufs=1,
                                      space="PSUM") as lnp:
                        ps_s = lnp.tile([1, V], f32, tag="lnsum")
                        ps_q = lnp.tile([1, V], f32, tag="lnsqsum")
                        for kc in range(4):
                            sq = lnt.tile([128, V], f32r, tag="lnsq", bufs=2,
                                          name="sq")
                            nc.gpsimd.tensor_mul(sq,
                                                 src_tiles[kc].bitcast(f32),
                                                 src_tiles[kc].bitcast(f32))
                            for nh in range(2):
                                s = slice(nh * 512, nh * 512 + 512)
                                mm(ps_s[:, s], ones, src_tiles[kc][:, s],
                                   start=(kc == 0), stop=(kc == 3))
                                mm(ps_q[:, s], ones, sq[:, s],
                                   start=(kc == 0), stop=(kc == 3))
                        m_row = lnt.tile([1, V], f32, tag="mrow")
                        nc.scalar.mul(m_row, ps_s, 1.0 / D)
                        msq = lnt.tile([1, V], f32, tag="msq")
                        nc.scalar.square(msq, m_row)
                        varr = lnt.tile([1, V], f32, tag="varr")
                        nc.vector.scalar_tensor_tensor(
                            varr, ps_q, 1.0 / D, msq, ALU.mult, ALU.subtract)
                    stdr = lnt.tile([1, V], f32, tag="stdr")
                    nc.scalar.activation(stdr, varr, ACTF.Sqrt, bias=epst)
                    with nc.allow_low_precision(
                            reason="1/std via fast approx"):
                        nc.vector.reciprocal_approx_fast(out=stdr, in_=stdr)
                    r_row = lnt.tile([1, V], f32r, tag="rrow")
                    nc.vector.tensor_copy(out=r_row, in_=stdr)
                    mrneg = lnt.tile([1, V], f32r, tag="mrneg")
                    nc.vector.scalar_tensor_tensor(
                        mrneg, m_row, -1.0, stdr,
                        ALU.mult, ALU.mult)
                    with tc.tile_pool(name="lnbp", bufs=2,
                                      space="PSUM") as lnbp:
                        for kc in range(4):
                            s1r = rows_params[0:1, base + kc * 128:
                                              base + kc * 128 + 128]
                            t1r = rows_params[0:1, base + 512 + kc * 128:
                                              base + 512 + kc * 128 + 128]
                            R1 = lnbp.tile([128, V], f32, tag="R1",
                                           name="R1")
                            Dt = lnbp.tile([128, V], f32, tag="Dt",
                                           name="Dt")
                            for nh in range(2):
                                s = slice(nh * 512, nh * 512 + 512)
                                mm(R1[:, s], s1r,
                                   r_row[:, s], start=True, stop=True)
                                mm(Dt[:, s], t1r,
                                   onesrow[:, s], start=True, stop=False)
                                mm(Dt[:, s], s1r,
                                   mrneg[:, s], start=False, stop=True)
                            u = lnt.tile([128, V], f32, tag="lnu", bufs=2,
                                         name="u")
                            nc.vector.tensor_mul(u, src_tiles[kc].bitcast(f32),
                                                 R1)
                            nc.vector.tensor_add(out[kc], u, Dt)
                return out

            # proj+mlp weights: pool outlives the attention scope; DMAs are
            # issued inside the attention loop so they stream during it
            mlpw = ctx.enter_context(tc.tile_pool(name="mlpw", bufs=1))
            wp_t = [mlpw.tile([128, 512], bf16, tag=f"wproj{kc}",
                              name="wp_t") for kc in range(4)]
            wm1_t = [mlpw.tile([128, 2048], bf16, tag=f"wm1{kc}",
                               name="wm1_t") for kc in range(4)]
            wm2_t = [mlpw.tile([128, 512], bf16, tag=f"wm2{kc}",
                               name="wm2_t") for kc in range(16)]

            # qk tiles (feature-major q then k), token-major v (+ones row)
            with tc.tile_pool(name="attlife", bufs=1) as attlife:
                qk = [attlife.tile([128, V], bf16, tag=f"qk{m}",
                                   name=f"qk{m}") for m in range(8)]
                vaug = [attlife.tile([128, 8, 65], bf16, tag=f"vaug{t}",
                                     name=f"vaug{t}") for t in range(8)]
                att = [attlife.tile([128, V], bf16, tag=f"att{kc}",
                                    name=f"att{kc}") for kc in range(4)]
                # Z rows packed [parity, kc*V + i]: head h -> row h%2,
                # free slice (h//2)*V
                zcat = attlife.tile([2, 4 * V], f32, tag="zcat")

                # h1 = AdaLN1(x); qk feature-major; v token-major
                with tc.tile_pool(name="h1pool", bufs=1) as h1pool:
                    h1 = adaln(xT_t, 0, h1pool, "h1")
                    with tc.tile_pool(name="qkvw", bufs=1) as qkvw, \
                            tc.tile_pool(name="qkvp", bufs=4,
                                         space="PSUM") as qkvp:
                        wqk_t = [qkvw.tile([128, 1024], bf16, tag=f"wqk{kc}",
                                           name="wqk_t") for kc in range(4)]
                        wv_t = [qkvw.tile([128, 512], bf16, tag=f"wv{kc}",
                                          name="wv_t") for kc in range(4)]
                        for kc in range(4):
                            nc.sync.dma_start(
                                out=wqk_t[kc],
                                in_=wqk[kc * 128:(kc + 1) * 128, :])
                            nc.sync.dma_start(
                                out=wv_t[kc],
                                in_=wv[kc * 128:(kc + 1) * 128, :])
                        # emit (q, k) tile pairs per head-pair so attention
                        # for early heads can start before qkv finishes
                        for m in (0, 4, 1, 5, 2, 6, 3, 7):
                            for nh in range(2):
                                s = slice(nh * 512, nh * 512 + 512)
                                pp = qkvp.tile([128, 512], f32, tag="mmqk")
                                for kc in range(4):
                                    mm(pp,
                                       wqk_t[kc][:, m * 128:(m + 1) * 128],
                                       h1[kc][:, s], start=(kc == 0),
                                       stop=(kc == 3))
                                nc.vector.tensor_copy(out=qk[m][:, s], in_=pp)
                        for t in range(8):
                            pp = qkvp.tile([128, 512], f32, tag="mmv")
                            for kc in range(4):
                                mm(pp, h1[kc][:, t * 128:(t + 1) * 128],
                                   wv_t[kc], start=(kc == 0), stop=(kc == 3))
                            nc.vector.tensor_copy(
                                out=vaug[t][:, :, 0:64],
                                in_=pp[:].rearrange("p (h d) -> p h d", h=8))
                            nc.gpsimd.memset(vaug[t][:, :, 64:65], 1.0)
                        # proj + mlp1 weights stream while qkv/attention
                        # compute; wm2 is issued at the end of the g stream
                        for kc in range(4):
                            nc.sync.dma_start(
                                out=wp_t[kc],
                                in_=wproj[kc * 128:(kc + 1) * 128, :])
                        for kc in range(4):
                            nc.sync.dma_start(
                                out=wm1_t[kc],
                                in_=wm1[kc * 128:(kc + 1) * 128, :])

                # attention: S^T[j,i]; P = exp(S) * g; softmax denom from
                # the vaug ones-row, normalized after the loop. Software-
                # pipelined with LA tiles of lookahead so the PE never
                # stalls on the exp->mul chain of the pair it just scored.
                LA = 3
                with tc.tile_pool(name="attt", bufs=1) as attt:
                  with tc.tile_pool(name="attps", bufs=LA,
                                    space="PSUM") as attps, \
                        tc.tile_pool(name="attpo", bufs=1,
                                     space="PSUM") as attpo:
                    for h in range(8):
                        ops = attpo.tile([65, V], f32, tag="ops",
                                         bufs=1, name="ops")
                        Ss = {}
                        gs = {}

                        def emit(h, jt):
                            g_t = attt.tile([128, V], bf16, tag="gt",
                                            bufs=2 * LA + 2, name="g_t")
                            nc.sync.dma_start(
                                out=g_t,
                                in_=gbias[(h * 8 + jt) * 128:
                                          (h * 8 + jt) * 128 + 128, :])
                            gs[jt] = g_t
                            kt = qk[4 + h // 2][
                                (h % 2) * 64:(h % 2) * 64 + 64,
                                jt * 128:jt * 128 + 128]
                            S = attps.tile([128, V], f32, tag="mms",
                                           name="S")
                            for nh in range(2):
                                s = slice(nh * 512, nh * 512 + 512)
                                qt = qk[h // 2][
                                    (h % 2) * 64:(h % 2) * 64 + 64, s]
                                mm(S[:, s], kt, qt, start=True, stop=True)
                            Ss[jt] = S

                        for jt in range(LA):
                            emit(h, jt)
                        for jt in range(8):
                            S = Ss.pop(jt)
                            g_t = gs.pop(jt)
                            P0 = attt.tile([128, V], bf16, tag="P0",
                                           bufs=3, name="P0")
                            nc.scalar.activation(P0, S, ACTF.Exp)
                            P = attt.tile([128, V], bf16, tag="P",
                                          bufs=3, name="P")
                            nc.vector.tensor_mul(P, P0, g_t)
                            for nh in range(2):
                                s = slice(nh * 512, nh * 512 + 512)
                                mm(ops[:, s], vaug[jt][:, h, :],
                                   P[:, s], start=(jt == 0),
                                   stop=(jt == 7))
                            if jt + LA < 8:
                                emit(h, jt + LA)
                            if h == 6 and jt == 0:
                                for kc in range(16):
                                    nc.sync.dma_start(
                                        out=wm2_t[kc],
                                        in_=wm2[kc * 128:(kc + 1) * 128, :])
                        ztmp = attt.tile([65, V], f32, tag="ztmp",
                                         bufs=2, name="ztmp")
                        nc.vector.tensor_copy(out=ztmp[64:65, :],
                                              in_=ops[64:65, :])
                        nc.sync.dma_start(
                            out=zcat[h % 2:h % 2 + 1,
                                     (h // 2) * V:(h // 2) * V + V],
                            in_=ztmp[64:65, :])
                        nc.vector.tensor_copy(
                            out=att[h // 2][(h % 2) * 64:
                                            (h % 2) * 64 + 64, :],
                            in_=ops[0:64, :])

                  # normalize: att[kc] rows 0-63 = head 2kc, 64-127 = 2kc+1
                  if True:
                    with nc.allow_low_precision(
                            reason="softmax denom reciprocal"):
                        nc.vector.reciprocal_approx_fast(out=zcat, in_=zcat)
                    rinv = attt.tile([2, 4 * V], bf16, tag="rinv")
                    nc.vector.tensor_copy(out=rinv, in_=zcat)
                    with tc.tile_pool(name="zbp", bufs=2,
                                      space="PSUM") as zbp:
                        for kc in range(4):
                            zb = zbp.tile([128, V], f32, tag="zb", name="zb")
                            for nh in range(2):
                                s = slice(nh * 512, nh * 512 + 512)
                                mm(zb[:, s], bandm,
                                   rinv[0:2, kc * V + nh * 512:
                                        kc * V + nh * 512 + 512],
                                   start=True, stop=True)
                            nc.vector.tensor_mul(att[kc], att[kc], zb)

                # proj + residual (in place into xT_t -> x2)
                with tc.tile_pool(name="projp", bufs=4,
                                  space="PSUM") as projp:
                    for m in range(4):
                        for nh in range(2):
                            s = slice(nh * 512, nh * 512 + 512)
                            pp = projp.tile([128, 512], f32, tag="mmproj")
                            for kc in range(4):
                                mm(pp, wp_t[kc][:, m * 128:(m + 1) * 128],
                                   att[kc][:, s], start=(kc == 0),
                                   stop=(kc == 3))
                            nc.vector.scalar_tensor_tensor(
                                xT_t[m][:, s], pp,
                                bp_t[:, m:m + 1],
                                xT_t[m][:, s].bitcast(f32), ALU.add,
                                ALU.add)

            # ---------- MLP branch (xT_t now holds x2) ----------
            with tc.tile_pool(name="mlplife", bufs=1) as mlplife:
                h2 = adaln(xT_t, 1, mlplife, "h2")
                with tc.tile_pool(name="mlpt", bufs=1) as mlpt, \
                        tc.tile_pool(name="mlpp", bufs=4,
                                     space="PSUM") as mlpp:
                    for nh in range(2):
                        s = slice(nh * 512, nh * 512 + 512)
                        gm = [mlpt.tile([128, 512], bf16, tag=f"gm{m}",
                                        name=f"gm{m}") for m in range(16)]
                        for m in range(16):
                            pp = mlpp.tile([128, 512], f32, tag="mmm1")
                            for kc in range(4):
                                mm(pp, wm1_t[kc][:, m * 128:(m + 1) * 128],
                                   h2[kc][:, s], start=(kc == 0),
                                   stop=(kc == 3))
                            if sim_mode:
                                sig = mlpt.tile([128, 512], f32, tag="gsig",
                                                bufs=2, name="gsig")
                                nc.scalar.activation(sig, pp, ACTF.Sigmoid,
                                                     scale=1.702)
                                nc.vector.tensor_mul(gm[m], pp, sig)
                            else:
                                nc.scalar.activation(gm[m], pp, ACTF.Gelu,
                                                     bias=bm1_t[:, m:m + 1])
                        for m in range(4):
                            pp = mlpp.tile([128, 512], f32, tag="mmm2")
                            for kc in range(16):
                                mm(pp, wm2_t[kc][:, m * 128:(m + 1) * 128],
                                   gm[kc], start=(kc == 0), stop=(kc == 15))
                            yt = mlpt.tile([128, 512], f32, tag="yt",
                                           bufs=2, name="yt")
                            nc.vector.scalar_tensor_tensor(
                                yt, pp, bm2_t[:, m:m + 1],
                                xT_t[m][:, s].bitcast(f32), ALU.add,
                                ALU.add)
                            nc.sync.dma_start(
                                out=yT[m * 128:(m + 1) * 128, s], in_=yt)

    nc.compile()
    return nc


def _make_in_maps(inputs):
    bf = ml_dtypes.bfloat16
    x = np.asarray(inputs["x"], dtype=np.float32)
    cond = np.asarray(inputs["cond"], dtype=np.float32)
    ei = np.asarray(inputs["edge_index"])
    w_qkv = np.asarray(inputs["w_qkv"], dtype=np.float32)
    et = np.asarray(inputs["edge_table"], dtype=np.float32)

    scale = 1.0 / np.sqrt(HD)
    wqk = w_qkv[:, :2 * D].copy()
    wqk[:, :D] *= scale
    wv = np.ascontiguousarray(w_qkv[:, 2 * D:])
    wada = np.concatenate([inputs["w_ada1"], inputs["w_ada2"]],
                          axis=1).astype(np.float32)
    badar = np.concatenate([inputs["b_ada1"], inputs["b_ada2"]]).astype(
        np.float32).copy()
    badar[:D] += 1.0          # fold the (1 + scale) into ada1 scale bias
    badar[2 * D:3 * D] += 1.0  # and ada2 scale bias

    etT = np.exp(et).T.astype(np.float32)  # [H, 4]

    shared = {
        "wqk": np.ascontiguousarray(wqk.astype(bf)),
        "wv": wv.astype(bf),
        "wada": np.ascontiguousarray(wada.astype(bf)),
        "badar": np.ascontiguousarray(badar.reshape(1, 2 * 2 * D)),
        "wproj": np.ascontiguousarray(
            inputs["w_proj"].astype(np.float32).astype(bf)),
        "bproj": np.ascontiguousarray(
            inputs["b_proj"].astype(np.float32).reshape(4, 128)),
        "wm1": np.ascontiguousarray(
            inputs["w_mlp1"].astype(np.float32).astype(bf)),
        "bm1": np.ascontiguousarray(
            inputs["b_mlp1"].astype(np.float32).reshape(16, 128)),
        "wm2": np.ascontiguousarray(
            inputs["w_mlp2"].astype(np.float32).astype(bf)),
        "bm2": np.ascontiguousarray(
            inputs["b_mlp2"].astype(np.float32).reshape(4, 128)),
        "onesd": np.ones((128, 1), dtype=np.float32),
        "onesrowd": np.ones((1, 1024), dtype=np.float32),
        "bandd": np.ascontiguousarray(np.concatenate([
            np.concatenate([np.ones(64), np.zeros(64)]),
            np.concatenate([np.zeros(64), np.ones(64)]),
        ]).reshape(2, 128).astype(bf)),
    }
    in_maps = []
    for b in range(B):
        # g[h, j, i] = exp(et[ei[i, j], h])
        g = etT[:, ei[b]]                      # [H, i, j]
        g = np.ascontiguousarray(g.transpose(0, 2, 1))  # [H, j, i]
        in_maps.append(dict(
            shared,
            xT=np.ascontiguousarray(x[b].T),
            condc=np.ascontiguousarray(cond[b].reshape(4, 128)),
            gbias=g.reshape(H * V, V).astype(bf),
        ))
    return in_maps


def kernel(**inputs):
    from concourse.bass_utils import run_bass_kernel_spmd

    if "prog" not in _cache:
        _cache["prog"] = _build_program()
    nc = _cache["prog"]

    in_maps = _make_in_maps(inputs)
    res = run_bass_kernel_spmd(nc, in_maps, core_ids=list(range(NCORES)))
    out = np.stack([np.ascontiguousarray(res.results[b]["yT"].T)
                    for b in range(B)])
    return out.astype(np.float32)
